# revision 5
# baseline (speedup 1.0000x reference)
"""CrossModalAttention Trainium2 kernel (8-core data parallel).

Reference (B,M,D,H = 65536,4,256,4; HD=SCORE_H=64):
  h = tanh(x@W1.T + b1); scores = h@W2.T + b2
  alpha = softmax(mask ? -inf : scores) over M
  q,k,v = x@W{q,k,v}.T + b; logits = q.k/8 with key-mask; p = softmax
  attn_out = (p@v)@Wo.T + bo; normed = LN(x+attn_out)*gamma + beta
  fused = sum_m alpha_m*(x_m + normed_m);  returns (fused, alpha)

Sharding: batch split across 8 NeuronCores (8192 samples each), weights
replicated.  Per core, superchunks of 512 samples (4 groups x 128
partitions), sample-major m-separated layout; global sample index within
a core is b = g*128 + p with g = sc*4 + s.

Matmuls run in fp32r (full-rate, ~1e-4 rel err); attention elementwise
runs in bf16 on the vector engine; LayerNorm/fusion in fp32.
Structurally-zero biases / identity affine params are skipped (detected
host-side; general paths exist for nonzero values).
"""

import numpy as np

import concourse.bacc as bacc
import concourse.mybir as mybir
from concourse import tile
from concourse.bass_utils import run_bass_kernel_spmd

f32 = mybir.dt.float32
f32r = mybir.dt.float32r
bf16 = mybir.dt.bfloat16
AF = mybir.ActivationFunctionType
ALU = mybir.AluOpType
AX = mybir.AxisListType

B, M, D, H, HD, SH = 65536, 4, 256, 4, 64, 64
NCORES = 8
NB = B // NCORES


def _build_nc(NB, cfg):
    add_qk_bias = cfg.get("add_qk_bias", False)   # bq (bk is softmax-invariant)
    add_b1 = cfg.get("add_b1", False)
    add_bo = cfg.get("add_bo", False)             # bo_eff = bo + bv@Wo.T (host-folded)
    add_affine = cfg.get("add_affine", False)     # gamma/beta
    assert NB % 512 == 0
    NSC = NB // 512
    NG = NB // 128

    nc = bacc.Bacc("TRN2", target_bir_lowering=False, debug=False,
                   num_devices=NCORES, enable_asserts=False)

    x_d = nc.dram_tensor("x", [NB, M, D], f32, kind="ExternalInput")
    maskneg_d = nc.dram_tensor("maskneg", [128, NG, M], f32, kind="ExternalInput")
    wqkvT_d = nc.dram_tensor("wqkvT", [D, 3 * D], f32, kind="ExternalInput")
    w1T_d = nc.dram_tensor("w1T", [D, SH], f32, kind="ExternalInput")
    w2_d = nc.dram_tensor("w2", [1, SH], f32, kind="ExternalInput")
    woT_d = nc.dram_tensor("woT", [D, D], f32, kind="ExternalInput")
    ident_d = nc.dram_tensor("ident", [128, 128], f32, kind="ExternalInput")
    bqbc_d = nc.dram_tensor("bqbc", [1, D], f32, kind="ExternalInput")
    b1bc_d = nc.dram_tensor("b1bc", [1, SH], f32, kind="ExternalInput")
    bobc_d = nc.dram_tensor("bobc", [1, D], f32, kind="ExternalInput")
    gam_d = nc.dram_tensor("gam", [1, D], f32, kind="ExternalInput")
    bet_d = nc.dram_tensor("bet", [1, D], f32, kind="ExternalInput")
    fused_d = nc.dram_tensor("fused", [NB, D], f32, kind="ExternalOutput")
    alpha_d = nc.dram_tensor("alphap", [128, NG, M], f32, kind="ExternalOutput")

    with tile.TileContext(nc) as tc:
        with (
            tc.tile_pool(name="weights", bufs=1) as wpool,
            tc.tile_pool(name="x", bufs=2) as xpool,
            tc.tile_pool(name="xt", bufs=6) as xtpool,
            tc.tile_pool(name="qkvb", bufs=2) as qkvpool,
            tc.tile_pool(name="at", bufs=2) as atpool,
            tc.tile_pool(name="ln", bufs=2) as lnpool,
            tc.tile_pool(name="scr", bufs=2) as scr,
            tc.tile_pool(name="ps_t", bufs=2, space="PSUM") as ps_t,
            tc.tile_pool(name="ps_qkv", bufs=2, space="PSUM") as ps_qkv,
            tc.tile_pool(name="ps_ao", bufs=2, space="PSUM") as ps_ao,
        ):
            # ---- resident weights / constants ----
            wqkvT = wpool.tile([128, 2, 3 * D], f32r, tag="wqkv")
            nc.sync.dma_start(wqkvT[:], wqkvT_d.ap().rearrange("(kb k) e -> k kb e", k=128).bitcast(f32r))
            w1T = wpool.tile([128, 2, SH], f32r, tag="w1")
            nc.sync.dma_start(w1T[:], w1T_d.ap().rearrange("(kb k) e -> k kb e", k=128).bitcast(f32r))
            woTf = wpool.tile([128, 2, D], f32, tag="wof")
            nc.sync.dma_start(woTf[:], woT_d.ap().rearrange("(kb k) e -> k kb e", k=128))
            woT = wpool.tile([128, 2, D], bf16, tag="wo")
            nc.vector.tensor_copy(woT[:], woTf[:])
            w2f = wpool.tile([128, SH], f32, tag="w2f")
            nc.sync.dma_start(w2f[:], w2_d.ap().partition_broadcast(128).squeeze(1))
            w2bc = wpool.tile([128, SH], bf16, tag="w2b")
            nc.vector.tensor_copy(w2bc[:], w2f[:])
            ident = wpool.tile([128, 128], f32, tag="ident")
            nc.sync.dma_start(ident[:], ident_d.ap())
            maskneg = wpool.tile([128, NG, M], f32, tag="maskneg")
            nc.sync.dma_start(maskneg[:], maskneg_d.ap())
            alpha_all = wpool.tile([128, NG, M], f32, tag="alpha")

            bqbc = b1bc = bobc = gambc = betbc = None
            if add_qk_bias:
                bqbc = wpool.tile([128, D], f32, tag="bqbc")
                nc.sync.dma_start(bqbc[:], bqbc_d.ap().partition_broadcast(128).squeeze(1))
            if add_b1:
                b1bc = wpool.tile([128, SH], f32, tag="b1bc")
                nc.sync.dma_start(b1bc[:], b1bc_d.ap().partition_broadcast(128).squeeze(1))
            if add_bo:
                bobc = wpool.tile([128, D], f32, tag="bobc")
                nc.sync.dma_start(bobc[:], bobc_d.ap().partition_broadcast(128).squeeze(1))
            if add_affine:
                gambc = wpool.tile([128, D], f32, tag="gambc")
                nc.sync.dma_start(gambc[:], gam_d.ap().partition_broadcast(128).squeeze(1))
                betbc = wpool.tile([128, D], f32, tag="betbc")
                nc.sync.dma_start(betbc[:], bet_d.ap().partition_broadcast(128).squeeze(1))

            for sc in range(NSC):
                xall = xpool.tile([128, 16, D], f32, tag="xall")
                for s in range(4):
                    g = sc * 4 + s
                    nc.sync.dma_start(xall[:, s * 4:(s + 1) * 4, :],
                                      x_d.ap()[g * 128:(g + 1) * 128, :, :])

                qbf = qkvpool.tile([128, 4, 4, D], bf16, tag="qbf")
                kbf = qkvpool.tile([128, 4, 4, D], bf16, tag="kbf")
                vbf = qkvpool.tile([128, 4, 4, D], bf16, tag="vbf")
                hact = qkvpool.tile([128, 4, 4, SH], bf16, tag="hact")

                # ---- transpose + projections + drains, per (s, m) ----
                for s in range(4):
                    for m in range(4):
                        sm = s * 4 + m
                        tps = ps_t.tile([128, 2, 128], f32, tag="tps")
                        for kb in range(2):
                            nc.tensor.transpose(tps[:, kb], xall[:, sm, kb * 128:(kb + 1) * 128], ident[:])
                        xt = xtpool.tile([128, 2, 128], f32r, tag="xt")
                        nc.scalar.copy(xt[:], tps[:])
                        qk = ps_qkv.tile([128, 832], f32, tag="qk")
                        for lo, hi in ((0, 256), (256, 512), (512, 768)):
                            for kb in range(2):
                                nc.tensor.matmul(qk[:, lo:hi], xt[:, kb], wqkvT[:, kb, lo:hi],
                                                 start=(kb == 0), stop=(kb == 1))
                        for kb in range(2):
                            nc.tensor.matmul(qk[:, 768:832], xt[:, kb], w1T[:, kb],
                                             start=(kb == 0), stop=(kb == 1))
                        if add_qk_bias:
                            qf = scr.tile([128, D], f32, tag="qbias")
                            nc.vector.tensor_add(qf[:], qk[:, 0:256], bqbc[:])
                            nc.vector.tensor_copy(qbf[:, s, m, :], qf[:])
                        else:
                            nc.scalar.copy(qbf[:, s, m, :], qk[:, 0:256])
                        nc.scalar.copy(kbf[:, s, m, :], qk[:, 256:512])
                        nc.scalar.copy(vbf[:, s, m, :], qk[:, 512:768])
                        if add_b1:
                            hf = scr.tile([128, SH], f32, tag="hbias")
                            nc.vector.tensor_add(hf[:], qk[:, 768:832], b1bc[:])
                            nc.scalar.activation(hact[:, s, m, :], hf[:], AF.Tanh)
                        else:
                            nc.scalar.activation(hact[:, s, m, :], qk[:, 768:832], AF.Tanh)

                # ---- scores + alpha (batched over s) ----
                scp = scr.tile([128, 16, SH], bf16, tag="scprod")
                nc.vector.tensor_mul(scp[:], hact[:].rearrange("p a b e -> p (a b) e"),
                                     w2bc[:].unsqueeze(1).broadcast_to((128, 16, SH)))
                scores = scr.tile([128, 16], f32, tag="scores")
                nc.vector.reduce_sum(scores[:], scp[:], axis=AX.X)
                scm = scr.tile([128, 4, 4], f32, tag="scm")
                nc.vector.tensor_add(scm[:], scores[:].rearrange("p (s m) -> p s m", s=4),
                                     maskneg[:, sc * 4:(sc + 1) * 4, :])
                smx = scr.tile([128, 4], f32, tag="smx")
                nc.vector.reduce_max(smx[:], scm[:], axis=AX.X)
                sce = scr.tile([128, 4, 4], f32, tag="sce")
                nc.vector.tensor_sub(sce[:], scm[:], smx[:].unsqueeze(2).broadcast_to((128, 4, 4)))
                nc.scalar.activation(sce[:], sce[:], AF.Exp)
                sse = scr.tile([128, 4], f32, tag="sse")
                nc.vector.reduce_sum(sse[:], sce[:], axis=AX.X)
                rse = scr.tile([128, 4], f32, tag="rse")
                nc.vector.reciprocal(rse[:], sse[:])
                nc.vector.tensor_mul(alpha_all[:, sc * 4:(sc + 1) * 4, :], sce[:],
                                     rse[:].unsqueeze(2).broadcast_to((128, 4, 4)))

                # ---- logits + key-mask + softmax over km ----
                # DVE ISA allows at most 3 free dims per AP: keep (h,hd)
                # merged in the muls and split per-s where a 4th dim appears.
                lg = scr.tile([128, 4, 16, 4], f32, tag="lg")  # [p, s, qm*km, h]
                for s in range(4):
                    prod = scr.tile([128, 16, 4, HD], bf16, tag="prod")  # [p,qm*km,h,hd]
                    prodv = prod[:].rearrange("p a h d -> p a (h d)").rearrange(
                        "p (q k) e -> p q k e", q=4)
                    nc.vector.tensor_mul(prodv,
                                         qbf[:, s].unsqueeze(2).broadcast_to((128, 4, 4, D)),
                                         kbf[:, s].unsqueeze(1).broadcast_to((128, 4, 4, D)))
                    nc.vector.reduce_sum(lg[:, s], prod[:], axis=AX.X)
                    mns = maskneg[:, sc * 4 + s, :]  # [128, 4] (km)
                    lgs = lg[:, s].rearrange("p (q k) h -> p q k h", q=4)
                    nc.vector.tensor_add(lgs, lgs,
                                         mns.unsqueeze(1).unsqueeze(3).broadcast_to((128, 4, 4, 4)))
                pmx = scr.tile([128, 4, 4, 4], f32, tag="pmx")    # [p,s,qm,h]
                pex = scr.tile([128, 4, 4, 4, 4], f32, tag="pex")  # [p,s,qm,h,km]
                for s in range(4):
                    lgp = lg[:, s].rearrange("p (q k) h -> p q h k", q=4)
                    nc.vector.reduce_max(pmx[:, s], lgp, axis=AX.X)
                    nc.vector.tensor_sub(pex[:, s], lgp,
                                         pmx[:, s].unsqueeze(3).broadcast_to((128, 4, 4, 4)))
                pexf = pex[:].rearrange("p s q h k -> p (s q h k)")
                nc.scalar.activation(pexf, pexf, AF.Exp)
                pse = scr.tile([128, 64], f32, tag="pse")   # [p, s*qm*h]
                nc.vector.reduce_sum(pse[:], pex[:].rearrange("p s q h k -> p (s q h) k"),
                                     axis=AX.X)
                pre = scr.tile([128, 64], f32, tag="pre")
                nc.vector.reciprocal(pre[:], pse[:])
                p_all = scr.tile([128, 4, 4, 4, 4], bf16, tag="pall")  # [p,s,qm,h,km]
                for s in range(4):
                    prs = pre[:, s * 16:(s + 1) * 16].rearrange("p (q h) -> p q h", q=4)
                    nc.vector.tensor_mul(p_all[:, s], pex[:, s],
                                         prs.unsqueeze(3).broadcast_to((128, 4, 4, 4)))

                # ---- attention + Wo + residual, per s ----
                yall = lnpool.tile([128, 16, D], f32, tag="yall")
                musum = scr.tile([128, 16], f32, tag="musum")
                for s in range(4):
                    attn = atpool.tile([128, 4, D], bf16, tag="attn")
                    tmp = atpool.tile([128, 4, D], bf16, tag="attntmp")
                    attn4 = attn[:].rearrange("p q (h d) -> p q h d", h=4)
                    tmp4 = tmp[:].rearrange("p q (h d) -> p q h d", h=4)
                    attnf = attn[:].rearrange("p q e -> p (q e)")
                    tmpf = tmp[:].rearrange("p q e -> p (q e)")
                    for km in range(4):
                        vv = (vbf[:, s, km, :].rearrange("p (h d) -> p h d", h=4)
                              .unsqueeze(1).broadcast_to((128, 4, 4, HD)))
                        pv = p_all[:, s, :, :, km].unsqueeze(3).broadcast_to((128, 4, 4, HD))
                        if km == 0:
                            nc.vector.tensor_mul(attn4, vv, pv)
                        else:
                            nc.vector.tensor_mul(tmp4, vv, pv)
                            nc.vector.tensor_add(attnf, attnf, tmpf)
                    attnT = atpool.tile([128, 2, 4, 128], bf16, tag="attnT")
                    for qm in range(4):
                        for eb in range(2):
                            nc.sync.dma_start(attnT[:, eb, qm, :],
                                              attn[:, qm, eb * 128:(eb + 1) * 128], transpose=True)
                    for qm in range(4):
                        sm = s * 4 + qm
                        ao = ps_ao.tile([128, D], f32, tag="ao")
                        for kb in range(2):
                            nc.tensor.matmul(ao[:], attnT[:, kb, qm, :], woT[:, kb],
                                             start=(kb == 0), stop=(kb == 1))
                        if add_bo:
                            aof = scr.tile([128, D], f32, tag="aof")
                            nc.vector.tensor_add(aof[:], ao[:], bobc[:])
                            nc.vector.scalar_tensor_tensor(
                                yall[:, sm, :], xall[:, sm, :], 0.0, aof[:],
                                op0=ALU.add, op1=ALU.add, accum_out=musum[:, sm:sm + 1])
                        else:
                            nc.vector.scalar_tensor_tensor(
                                yall[:, sm, :], xall[:, sm, :], 0.0, ao[:],
                                op0=ALU.add, op1=ALU.add, accum_out=musum[:, sm:sm + 1])

                # ---- LN smalls (batched [128,16]) ----
                negmu = scr.tile([128, 16], f32, tag="negmu")
                nc.vector.tensor_scalar_mul(negmu[:], musum[:], -1.0 / D)
                varsum = scr.tile([128, 16], f32, tag="varsum")
                sq = scr.tile([128, D], f32, tag="sq")
                for sm in range(16):
                    nc.scalar.activation(sq[:], yall[:, sm, :], AF.Square,
                                         bias=negmu[:, sm:sm + 1], accum_out=varsum[:, sm:sm + 1])
                var = scr.tile([128, 16], f32, tag="var")
                nc.vector.tensor_scalar(var[:], varsum[:], 1.0 / D, 1e-5, op0=ALU.mult, op1=ALU.add)
                stdv = scr.tile([128, 16], f32, tag="stdv")
                nc.scalar.activation(stdv[:], var[:], AF.Sqrt)
                rstd = scr.tile([128, 16], f32, tag="rstd")
                nc.vector.reciprocal(rstd[:], stdv[:])
                alsc = alpha_all[:, sc * 4:(sc + 1) * 4, :].rearrange("p s m -> p (s m)")
                scl = scr.tile([128, 16], f32, tag="scl")
                nc.vector.tensor_mul(scl[:], rstd[:], alsc)
                nbias = scr.tile([128, 16], f32, tag="nbias")
                nc.vector.tensor_mul(nbias[:], negmu[:], scl[:])

                # ---- fusion: fused = sum_m alpha*(x + normed) ----
                for s in range(4):
                    g = sc * 4 + s
                    fz = lnpool.tile([128, 8, D], f32, tag="fz")
                    for m in range(4):
                        sm = s * 4 + m
                        if add_affine:
                            nrm = scr.tile([128, D], f32, tag="nrm")
                            nc.scalar.activation(nrm[:], yall[:, sm, :], AF.Identity,
                                                 bias=nbias[:, sm:sm + 1], scale=scl[:, sm:sm + 1])
                            nc.vector.tensor_mul(nrm[:], nrm[:], gambc[:])
                            nc.vector.scalar_tensor_tensor(
                                fz[:, 4 + m, :], betbc[:], alpha_all[:, g, m].unsqueeze(1), nrm[:],
                                op0=ALU.mult, op1=ALU.add)
                        else:
                            nc.scalar.activation(fz[:, 4 + m, :], yall[:, sm, :], AF.Identity,
                                                 bias=nbias[:, sm:sm + 1], scale=scl[:, sm:sm + 1])
                        nc.scalar.mul(fz[:, m, :], xall[:, sm, :],
                                      alpha_all[:, g, m].unsqueeze(1))
                    l1 = scr.tile([128, 4, D], f32, tag="l1")
                    nc.vector.tensor_add(l1[:], fz[:, 0:4, :], fz[:, 4:8, :])
                    l2 = scr.tile([128, 2, D], f32, tag="l2")
                    nc.vector.tensor_add(l2[:], l1[:, 0:2, :], l1[:, 2:4, :])
                    fo = scr.tile([128, D], f32, tag="fo")
                    nc.vector.tensor_add(fo[:], l2[:, 0, :], l2[:, 1, :])
                    nc.sync.dma_start(fused_d.ap()[g * 128:(g + 1) * 128, :], fo[:])

            nc.sync.dma_start(alpha_d.ap(), alpha_all[:])
    nc.compile()
    return nc


def _prep_weights(inputs):
    Wq = np.asarray(inputs["Wq"], np.float32)
    Wk = np.asarray(inputs["Wk"], np.float32)
    Wv = np.asarray(inputs["Wv"], np.float32)
    Wo = np.asarray(inputs["Wo"], np.float32)
    W1 = np.asarray(inputs["W1"], np.float32)
    W2 = np.asarray(inputs["W2"], np.float32)
    bq = np.asarray(inputs["bq"], np.float32)
    bv = np.asarray(inputs["bv"], np.float32)
    bo = np.asarray(inputs["bo"], np.float32)
    b1 = np.asarray(inputs["b1"], np.float32)
    gam = np.asarray(inputs["gamma"], np.float32)
    bet = np.asarray(inputs["beta"], np.float32)

    scale = np.float32(1.0 / np.sqrt(np.float32(HD)))
    wqkvT = np.concatenate([(Wq * scale).T, Wk.T, Wv.T], axis=1)
    bo_eff = bo + bv @ Wo.T

    cfg = {
        "add_qk_bias": bool(np.any(bq != 0)),
        "add_b1": bool(np.any(b1 != 0)),
        "add_bo": bool(np.any(bo_eff != 0)),
        "add_affine": bool(np.any(gam != 1) or np.any(bet != 0)),
    }
    params = {
        "wqkvT": np.ascontiguousarray(wqkvT, np.float32),
        "w1T": np.ascontiguousarray(W1.T, np.float32),
        "w2": np.ascontiguousarray(W2.reshape(1, SH), np.float32),
        "woT": np.ascontiguousarray(Wo.T, np.float32),
        "ident": np.eye(128, dtype=np.float32),
        "bqbc": (bq * scale).reshape(1, D).astype(np.float32),
        "b1bc": b1.reshape(1, SH).astype(np.float32),
        "bobc": bo_eff.reshape(1, D).astype(np.float32),
        "gam": gam.reshape(1, D).astype(np.float32),
        "bet": bet.reshape(1, D).astype(np.float32),
    }
    return params, cfg


_NC_CACHE = {}


def _get_nc(NB, cfg):
    key = (NB, tuple(sorted(cfg.items())))
    if key not in _NC_CACHE:
        _NC_CACHE[key] = _build_nc(NB, cfg)
    return _NC_CACHE[key]


def _run(inputs, trace=False, tmpdir=None):
    x = np.ascontiguousarray(np.asarray(inputs["modality_features"], np.float32))
    mask = np.asarray(inputs["modality_mask"], bool)
    Btot = x.shape[0]
    assert Btot % NCORES == 0
    nb = Btot // NCORES
    ng = nb // 128

    params, cfg = _prep_weights(inputs)
    nc = _get_nc(nb, cfg)

    in_maps = []
    for c in range(NCORES):
        xs = x[c * nb:(c + 1) * nb]
        ms = mask[c * nb:(c + 1) * nb]
        mn = np.where(ms, np.float32(-1e30), np.float32(0.0))
        mn = np.ascontiguousarray(mn.reshape(ng, 128, M).transpose(1, 0, 2))
        in_maps.append({"x": xs, "maskneg": mn, **params})

    try:
        res = run_bass_kernel_spmd(nc, in_maps, list(range(NCORES)),
                                   trace=trace, tmpdir=tmpdir)
    except ModuleNotFoundError:
        # NTFF profile hook unavailable in this environment; run untraced.
        res = run_bass_kernel_spmd(nc, in_maps, list(range(NCORES)), trace=False)
    fused = np.concatenate([r["fused"] for r in res.results], axis=0)
    alpha = np.concatenate(
        [r["alphap"].transpose(1, 0, 2).reshape(nb, M) for r in res.results], axis=0)
    return (fused, alpha), res


def kernel(**inputs):
    (fused, alpha), _ = _run(inputs, trace=False)
    return fused, alpha


def kernel_traced(**inputs):
    """Like kernel() but captures an NTFF profile; returns (outs, results)."""
    import tempfile
    td = tempfile.mkdtemp(prefix="cmattn_trace_")
    return _run(inputs, trace=True, tmpdir=td) + (td,)
